# revision 1
# baseline (speedup 1.0000x reference)
"""CrossModalTripletLoss kernel for 8 Trainium2 NeuronCores.

Strategy (data-parallel over the batch dim, 512 rows per core):

The reference samples ERROR_NUM=4 random negatives per row by taking
top_k over `jax.random.uniform(key=42)` scores masked to label-disjoint
pairs.  The random scores are *input-independent* constants, so the
per-row candidate order (score descending, ties -> lower index, exactly
jax.lax.top_k semantics) is precomputed on the host once.  Only the
first M=12 candidates per row are kept: the top-4 *unmasked* candidates
fall inside the first 12 with probability 1 - ~2e-12 for any plausible
label distribution (labels are one-hot over 80 classes, so ~1.3% of a
row's candidates are masked).

The host stages (pure data movement, no arithmetic on input values):
per-core row slices, and the candidate LABEL rows permuted by the
constant index tables (labels[c[i,m]] -> lab_g), because the HW
indirect DMA consumes exactly one offset per partition per call.

On device (raw Bass, explicit semaphores; Tile is unusable with this
walrus build, which accepts at most one embedded sync wait per
instruction), per 128-row chunk and per modality:
  1. inter[i,c] = <lab_i, lab_cand>            (DVE mult + 3D reduce)
  2. val = (inter == 0) * pcode, pcode = (16-c)*4096 + cand_col —
     priority and column index packed in one exact-int f32
  3. vector.max -> top-8; the first 4 are the selected negatives in
     exactly the reference's top_k order; column = int(val) & 0xFFF
  4. GPSIMD indirect-DMA gathers the 4 selected embedding rows
     (one call per slot: one offset per partition per call)
  5. dist = sqrt(sum((own - cand)^2)): DVE sub, ACT square, DVE reduce
  6. relu(pos - neg + margin) on ACT with per-partition bias = pos+1
The DVE stream is software-pipelined in three stages so selection for
chunk-mod t+1 runs while t's gather is in flight; squares run on the
otherwise-idle ACT engine.  Per-core output is a [128,1] vector of
per-partition loss-term sums; the host adds the 8x128 partials and
divides by B*ERROR_NUM.
"""

import sys

import numpy as np

for _p in ("/opt/trn_rl_repo",):
    if _p not in sys.path:
        sys.path.insert(0, _p)

B, D, C = 4096, 128, 80
NCORES = 8
RPC = B // NCORES          # rows per core = 512
P = 128                    # partitions
NCHUNK = RPC // P          # 4 chunks of 128 rows per core
M = 12                     # candidates kept per row (top-4 unmasked fall in
                           # the first 12 w.p. 1-~2e-12 for one-hot C=80 labels)
K = 4                      # ERROR_NUM
MARGIN = 1.0
ENC = 4096.0               # priority encoding multiplier

_CACHE = {}


def _host_tables():
    """Constant candidate tables from the reference's fixed RNG key 42."""
    if "pc" in _CACHE:
        return _CACHE["pc"]
    import jax

    # Replicate the reference's RNG calls exactly: default PRNG impl and
    # default device (rbg output is backend-dependent, so no pinning).
    skey = jax.random.key(42)
    ks1, ks2 = jax.random.split(skey)
    u1 = np.asarray(jax.random.uniform(ks1, (B, B)))
    u2 = np.asarray(jax.random.uniform(ks2, (B, B)))
    # candidate order = top_k order: value desc, ties -> lower index
    c1 = np.argsort(-u1, axis=1, kind="stable")[:, :M].astype(np.int32)
    c2 = np.argsort(-u2, axis=1, kind="stable")[:, :M].astype(np.int32)
    prio = (M - np.arange(M)).astype(np.float32)  # M .. 1
    pc1 = prio[None, :] * ENC + c1.astype(np.float32)
    pc2 = prio[None, :] * ENC + c2.astype(np.float32)
    _CACHE["pc"] = (c1, c2, pc1, pc2)
    return _CACHE["pc"]


def _build_nc(nrep=1):
    key = ("nc", nrep)
    if key in _CACHE:
        return _CACHE[key]
    from contextlib import ExitStack

    import concourse.bass as bass
    import concourse.mybir as mybir

    f32 = mybir.dt.float32
    i32 = mybir.dt.int32
    Alu = mybir.AluOpType
    Act = mybir.ActivationFunctionType
    X = mybir.AxisListType.X

    nc = bass.Bass()
    own_pack = nc.declare_dram_parameter(
        "own_pack", [RPC, 2 * D + C], f32, isOutput=False
    )
    txt_full = nc.declare_dram_parameter("txt_full", [B, D], f32, isOutput=False)
    img_full = nc.declare_dram_parameter("img_full", [B, D], f32, isOutput=False)
    pcode_d = nc.declare_dram_parameter(
        "pcode", [2, NCHUNK, P, M], f32, isOutput=False
    )
    lab_g = nc.declare_dram_parameter(
        "lab_g", [2, NCHUNK, P, M * C], f32, isOutput=False
    )
    partial = nc.declare_dram_parameter("partial", [P, 1], f32, isOutput=True)
    dbg = nc.declare_dram_parameter(
        "dbg", [P, 2 * NCHUNK * K], f32, isOutput=True
    )

    es = ExitStack()
    _n = [0]

    def sb(shape, dt=f32, name=None):
        _n[0] += 1
        nm = name or f"t{_n[0]}"
        return es.enter_context(nc.sbuf_tensor(nm, shape, dt))

    own = [sb([P, 2 * D + C]) for _ in range(4)]
    sel = [sb([P, M]) for _ in range(4)]
    labg = [sb([P, M * C]) for _ in range(4)]
    embg = [sb([P, K * D]) for _ in range(4)]
    coli = [sb([P, K], i32) for _ in range(4)]
    codei = sb([P, K], i32)
    dif = sb([P, D])
    sqp = sb([P, D])
    prod = sb([P, M * C])
    inter = sb([P, M])
    val = sb([P, M])
    top8 = sb([P, 8])
    dif4 = [sb([P, K * D]) for _ in range(2)]
    sq4 = [sb([P, K * D]) for _ in range(2)]
    trash4 = sb([P, K * D])
    pd2 = [sb([P, 1]) for _ in range(4)]
    posb = [sb([P, 1]) for _ in range(4)]
    posb1 = [sb([P, 1]) for _ in range(4)]
    nd2 = [sb([P, K]) for _ in range(4)]
    negd = sb([P, K])
    collect = sb([P, 2 * NCHUNK * K])
    red = sb([P, 1])

    def sem(nm):
        return es.enter_context(nc.semaphore(nm))

    s_own = [sem(f"s_own{i}") for i in range(4)]
    s_sel = [sem(f"s_sel{i}") for i in range(4)]
    s_labg = [sem(f"s_labg{i}") for i in range(4)]
    s_embg = [sem(f"s_embg{i}") for i in range(4)]
    s_prod, s_val, s_coli = sem("s_prod"), sem("s_val"), sem("s_coli")
    s_dif4, s_nd2, s_pd2 = sem("s_dif4"), sem("s_nd2"), sem("s_pd2")
    s_relu, s_red, s_out = sem("s_relu"), sem("s_red"), sem("s_out")
    s_psq, s_sq4 = sem("s_psq"), sem("s_sq4")

    with es, nc.Block() as block:

        @block.sync
        def _(sync):
            for g in range(NCHUNK * nrep):
                gc = g % NCHUNK
                if g >= 4:
                    sync.wait_ge(s_dif4, 2 * g - 6)
                sync.dma_start(
                    own[g % 4][:], own_pack[gc * P : (gc + 1) * P, :]
                ).then_inc(s_own[g % 4], 16)
                for mod in range(2):
                    cm = 2 * g + mod
                    if cm >= 4:
                        sync.wait_ge(s_val, cm - 3)
                    sync.dma_start(sel[cm % 4][:], pcode_d[mod, gc]).then_inc(
                        s_sel[cm % 4], 16
                    )
                    if cm >= 4:
                        sync.wait_ge(s_prod, cm - 3)
                    sync.dma_start(labg[cm % 4][:], lab_g[mod, gc]).then_inc(
                        s_labg[cm % 4], 16
                    )
            sync.wait_ge(s_red, 1)
            sync.dma_start(partial[:, :], red[:]).then_inc(s_out, 16)
            sync.dma_start(dbg[:, :], collect[:]).then_inc(s_out, 16)

        @block.gpsimd
        def _(gpsimd):
            for cm in range(2 * NCHUNK * nrep):
                full_emb = txt_full if cm % 2 == 0 else img_full
                gpsimd.wait_ge(s_coli, cm + 1)
                if cm >= 4:
                    gpsimd.wait_ge(s_dif4, cm - 3)
                for k in range(K):
                    # HW indirect DMA consumes one offset per partition and
                    # gathers out.free_size contiguous elements, so each
                    # selected row needs its own call.
                    gpsimd.indirect_dma_start(
                        out=embg[cm % 4][:, k * D : (k + 1) * D],
                        out_offset=None,
                        in_=full_emb[:],
                        in_offset=bass.IndirectOffsetOnAxis(
                            ap=coli[cm % 4][:, k : k + 1], axis=0
                        ),
                    ).then_inc(s_embg[cm % 4], 16)

        @block.vector
        def _(vector):
            def stage_a(cm):
                g = cm // 2
                mod = cm % 2
                oi = own[g % 4][:, 0:D]
                ot = own[g % 4][:, D : 2 * D]
                ol = own[g % 4][:, 2 * D : 2 * D + C]
                if mod == 0:
                    vector.wait_ge(s_own[g % 4], 16 * (g // 4 + 1))
                    if g >= 4:
                        vector.wait_ge(s_relu, 2 * g - 6)
                    nc.vector.tensor_tensor(
                        out=dif[:], in0=oi, in1=ot, op=Alu.subtract
                    )
                    vector.drain()
                    nc.vector.tensor_tensor(
                        out=sqp[:], in0=dif[:], in1=dif[:], op=Alu.mult
                    )
                    vector.drain()
                    nc.vector.tensor_reduce(
                        out=pd2[g % 4][:], in_=sqp[:], axis=X, op=Alu.add
                    ).then_inc(s_pd2, 1)
                pcd = sel[cm % 4][:]
                vector.wait_ge(s_labg[cm % 4], 16 * (cm // 4 + 1))
                vector.wait_ge(s_sel[cm % 4], 16 * (cm // 4 + 1))
                nc.vector.tensor_tensor(
                    out=prod[:].rearrange("p (m c) -> p m c", c=C),
                    in0=labg[cm % 4][:].rearrange("p (m c) -> p m c", c=C),
                    in1=ol.unsqueeze(1).broadcast_to([P, M, C]),
                    op=Alu.mult,
                ).then_inc(s_prod, 1)
                vector.drain()
                nc.vector.tensor_reduce(
                    out=inter[:],
                    in_=prod[:].rearrange("p (m c) -> p m c", c=C),
                    axis=X,
                    op=Alu.add,
                )
                vector.drain()
                nc.vector.scalar_tensor_tensor(
                    out=val[:],
                    in0=inter[:],
                    scalar=0.0,
                    in1=pcd,
                    op0=Alu.is_equal,
                    op1=Alu.mult,
                ).then_inc(s_val, 1)
                vector.drain()
                nc.vector.max(out=top8[:], in_=val[:])
                vector.drain()
                nc.vector.tensor_copy(out=codei[:], in_=top8[:, :K])
                vector.drain()
                if cm >= 4:
                    vector.wait_ge(s_embg[cm % 4], 64 * (cm // 4))
                nc.vector.tensor_scalar(
                    out=coli[cm % 4][:],
                    in0=codei[:],
                    scalar1=4095,
                    scalar2=None,
                    op0=Alu.bitwise_and,
                ).then_inc(s_coli, 1)

            def stage_b(cm):
                g = cm // 2
                mod = cm % 2
                oi = own[g % 4][:, 0:D]
                ot = own[g % 4][:, D : 2 * D]
                own_emb = oi if mod == 0 else ot
                vector.wait_ge(s_embg[cm % 4], 64 * (cm // 4 + 1))
                if cm >= 2:
                    vector.wait_ge(s_sq4, cm - 1)
                nc.vector.tensor_tensor(
                    out=dif4[cm % 2][:].rearrange("p (k d) -> p k d", d=D),
                    in0=embg[cm % 4][:].rearrange("p (k d) -> p k d", d=D),
                    in1=own_emb.unsqueeze(1).broadcast_to([P, K, D]),
                    op=Alu.subtract,
                ).then_inc(s_dif4, 1)

            def stage_c(cm):
                vector.wait_ge(s_sq4, cm + 1)
                if cm >= 4:
                    vector.wait_ge(s_relu, cm - 3)
                nc.vector.tensor_reduce(
                    out=nd2[cm % 4][:],
                    in_=sq4[cm % 2][:].rearrange("p (k d) -> p k d", d=D),
                    axis=X,
                    op=Alu.add,
                ).then_inc(s_nd2, 1)

            TOT = 2 * NCHUNK * nrep
            LAG = 2
            for t in range(TOT + LAG + 1):
                if t < TOT:
                    stage_a(t)
                if LAG <= t < TOT + LAG:
                    stage_b(t - LAG)
                if t >= LAG + 1:
                    stage_c(t - LAG - 1)
            vector.wait_ge(s_relu, 2 * NCHUNK * nrep)
            nc.vector.tensor_reduce(
                out=red[:], in_=collect[:], axis=X, op=Alu.add
            ).then_inc(s_red, 1)

        @block.scalar
        def _(scalar):
            def act_pos(g):
                scalar.wait_ge(s_pd2, g + 1)
                nc.scalar.activation(
                    out=posb[g % 4][:], in_=pd2[g % 4][:], func=Act.Sqrt
                )
                scalar.drain()
                nc.scalar.activation(
                    out=posb1[g % 4][:],
                    in_=posb[g % 4][:],
                    func=Act.Identity,
                    bias=MARGIN,
                )
                scalar.drain()

            def act_sq(cm):
                scalar.wait_ge(s_dif4, cm + 1)
                if cm >= 2:
                    scalar.wait_ge(s_nd2, cm - 1)
                nc.scalar.activation(
                    out=sq4[cm % 2][:], in_=dif4[cm % 2][:], func=Act.Square
                ).then_inc(s_sq4, 1)
                scalar.drain()

            def act_tail(cm):
                g = cm // 2
                scalar.wait_ge(s_nd2, cm + 1)
                nc.scalar.activation(
                    out=negd[:], in_=nd2[cm % 4][:], func=Act.Sqrt
                )
                scalar.drain()
                nc.scalar.activation(
                    out=collect[:, K * (cm % (2 * NCHUNK)) : K * (cm % (2 * NCHUNK)) + K],
                    in_=negd[:],
                    func=Act.Relu,
                    scale=-1.0,
                    bias=posb1[g % 4][:],
                ).then_inc(s_relu, 1)
                scalar.drain()

            TOT = 2 * NCHUNK * nrep
            for t in range(TOT + 2):
                if t < TOT:
                    if t % 2 == 0:
                        act_pos(t // 2)
                    act_sq(t)
                if t >= 2:
                    act_tail(t - 2)

    _CACHE[key] = nc
    return nc


def make_in_maps(image_hash, text_hash, labels):
    image_hash = np.ascontiguousarray(image_hash, dtype=np.float32)
    text_hash = np.ascontiguousarray(text_hash, dtype=np.float32)
    labels = np.ascontiguousarray(labels, dtype=np.float32)
    c1, c2, pc1, pc2 = _host_tables()
    in_maps = []
    for m in range(NCORES):
        rs = slice(m * RPC, (m + 1) * RPC)
        pcode = np.empty((2, NCHUNK, P, M), np.float32)
        labg = np.empty((2, NCHUNK, P, M * C), np.float32)
        for mod, (cc, pp) in enumerate(((c1, pc1), (c2, pc2))):
            pcode[mod] = pp[rs].astype(np.float32).reshape(NCHUNK, P, M)
            labg[mod] = labels[cc[rs]].reshape(NCHUNK, P, M * C)
        in_maps.append(
            {
                "own_pack": np.concatenate(
                    [image_hash[rs], text_hash[rs], labels[rs]], axis=1
                ),
                "txt_full": text_hash,
                "img_full": image_hash,
                "pcode": pcode,
                "lab_g": labg,
            }
        )
    return in_maps


def run_kernel(image_hash, text_hash, labels, trace=False, **kw):
    from concourse.bass_utils import run_bass_kernel_spmd

    nc = _build_nc()
    in_maps = make_in_maps(image_hash, text_hash, labels)
    res = run_bass_kernel_spmd(
        nc, in_maps, list(range(NCORES)), trace=trace, **kw
    )
    total = 0.0
    for r in res.results:
        total += float(np.asarray(r["partial"], dtype=np.float64).sum())
    loss = np.float32(total / (B * K))
    return loss, res


def kernel(image_hash, text_hash, labels):
    loss, _ = run_kernel(image_hash, text_hash, labels)
    return np.asarray(loss, dtype=np.float32)

